# revision 15
# baseline (speedup 1.0000x reference)
"""Distributed ISTFT kernel for Trainium2 (8 NeuronCores, Bass/Tile).

Math (matches the jax reference):
  z: [2, 513, T] one-sided spectrum (real/imag), T = 8192 frames.
  Hermitian extension + ifft(1024) + window + overlap-add (hop 256) +
  divide by overlapped window sum + trim 512 each side -> [2, 2096896].

Key folds used here:
  * real(ifft) = A^T @ X where A [1024(k), 1024(n)] packs the cos rows for
    zr bins 0..512 and sin rows for zi bins 1..511; X packs those z rows.
  * imag(ifft)[n, t] = (zi[0,t] + (-1)^n zi[512,t]) / N  (rank-2).
  * Output sample m = 256*b + r; block b = sum_{q=0..3} wf_{b-q}[256q+r].
    Folding window * A into the stationary operand gives
    O^T[t, r] = sum_q X[:, t+3-q]^T @ Aw_q directly -- the overlap-add and
    windowing ride inside the matmul.
  * The overlapped window sum for the periodic Hann window at hop N/4 is
    EXACTLY 2.0 everywhere except the first/last 256 output samples, so
    the 1/ws normalization folds into A as a global *0.5 (and into the
    ch1 taps); the two edge blocks are rescaled on the host (512 samples
    per channel, elementwise).
  * Everything streams in bf16 (tolerance is 2e-2; achieved ~3e-3),
    which halves HBM traffic and enables FWL on the PE.
  * Channel 1 (rank-2) runs FIRST with host-pre-shifted zi0/zi512 rows,
    doubling as the PE HAM warm-up while the big streams land.
  * Frame axis is sharded 1024 output blocks/core with a 3-frame input
    halo, so no cross-core communication is needed at all.
"""

import numpy as np
import ml_dtypes

N_FFT = 1024
HOP = 256
T_FRAMES = 8192
N_CORES = 8
F_SLOTS = 1027  # frame slots per core: 1024 owned blocks need slots t..t+3
NB = 1024       # output blocks computed per core (core 7 uses 1023)

BF16 = ml_dtypes.bfloat16

_CACHE = {}


def _amat() -> np.ndarray:
    """A [1024(kappa), 1024(n)]: ifft cos/sin weights, f64 (pre-fold)."""
    n = np.arange(N_FFT, dtype=np.float64)[None, :]
    k = np.arange(513, dtype=np.float64)[:, None]
    g = np.full((513, 1), 2.0)
    g[0, 0] = 1.0
    g[512, 0] = 1.0
    C = (g / N_FFT) * np.cos(2.0 * np.pi * k * n / N_FFT)
    k2 = np.arange(1, 512, dtype=np.float64)[:, None]
    S = (-2.0 / N_FFT) * np.sin(2.0 * np.pi * k2 * n / N_FFT)
    return np.concatenate([C, S], 0)


def _build_nc():
    from contextlib import ExitStack

    import concourse.tile as tile
    from concourse import bacc, mybir

    f32 = mybir.dt.float32
    bf = mybir.dt.bfloat16

    nc = bacc.Bacc("TRN2", target_bir_lowering=False, debug=False,
                   num_devices=N_CORES)

    # x rows: 0..512 zr, 513..1023 zi1..511, 1024 zi0, 1025 zi512,
    # 1026..1029 tu (zi0 pre-shifted), 1030..1033 tv (zi512 pre-shifted)
    x_d = nc.dram_tensor("x", [1034, F_SLOTS], bf, kind="ExternalInput")
    a_d = nc.dram_tensor("awn", [1024, 1024], bf, kind="ExternalInput")
    t_d = nc.dram_tensor("taps", [8, 256], bf, kind="ExternalInput")
    o_d = nc.dram_tensor("out", [2, NB, 256], bf, kind="ExternalOutput")

    with tile.TileContext(nc) as tc, ExitStack() as ctx:
        big = ctx.enter_context(tc.tile_pool(name="big", bufs=1))
        sml = ctx.enter_context(tc.tile_pool(name="sml", bufs=1))
        osb = ctx.enter_context(tc.tile_pool(name="osb", bufs=8))

        # small inputs on the gpsimd (SWDGE) queue
        tpu = sml.tile([4, 256], bf, tag="tpu")
        nc.gpsimd.dma_start(out=tpu[:], in_=t_d.ap()[0:4, :])
        tpv = sml.tile([4, 256], bf, tag="tpv")
        nc.gpsimd.dma_start(out=tpv[:], in_=t_d.ap()[4:8, :])

        # big streams: x chunks on sync, Aw chunks on scalar (both HWDGE);
        # the small ch1 lhsT tiles ride early on the same queues.
        xs = []
        tut = sml.tile([4, NB], bf, tag="tut")
        tvt = sml.tile([4, NB], bf, tag="tvt")
        for k in range(8):
            xk = big.tile([128, F_SLOTS], bf, tag=f"xs{k}")
            nc.sync.dma_start(out=xk[:],
                              in_=x_d.ap()[128 * k:128 * (k + 1), :])
            xs.append(xk)
            if k == 0:
                nc.sync.dma_start(out=tut[:], in_=x_d.ap()[1026:1030, 0:NB])
        aw = []
        for k in range(8):
            ak = big.tile([128, N_FFT], bf, tag=f"aw{k}")
            nc.scalar.dma_start(out=ak[:],
                                in_=a_d.ap()[128 * k:128 * (k + 1), :])
            aw.append(ak)
            if k == 0:
                nc.scalar.dma_start(out=tvt[:], in_=x_d.ap()[1030:1034, 0:NB])

        # HAM warm-up: dummy matmuls on a memset tile sized to end right
        # when the first data chunks land (~10.5us), so the PE clock-gate
        # flips to 8/8 just as the real stream begins and never re-cools.
        ps0p = ctx.enter_context(tc.tile_pool(name="ps0p", bufs=6,
                                              space="PSUM"))
        ps1p = ctx.enter_context(tc.tile_pool(name="ps1p", bufs=2,
                                              space="PSUM"))
        wtile = sml.tile([4, 256], bf, tag="wtile")
        nc.vector.memset(wtile[:], 0.0)
        wps = ps1p.tile([128, 256], f32, tag="ps1", name="warm")
        for i in range(17):
            nc.tensor.matmul(wps[:], lhsT=wtile[:, 0:128], rhs=wtile[:],
                             start=(i == 0), stop=(i == 16))

        def evict(ps, tt, ch, queue):
            o = osb.tile([128, 256], bf, tag=f"o{ch}", name=f"o{ch}_{tt}")
            nc.vector.tensor_copy(o[:], ps[:])
            queue.dma_start(
                out=o_d.ap()[ch:ch + 1, tt * 128:(tt + 1) * 128, :], in_=o[:])

        def ch1_group(tt):
            ps1 = ps1p.tile([128, 256], f32, tag="ps1", name=f"ps1_{tt}")
            nc.tensor.matmul(ps1[:], lhsT=tut[:, tt * 128:tt * 128 + 128],
                             rhs=tpu[:], start=True, stop=False)
            nc.tensor.matmul(ps1[:], lhsT=tvt[:, tt * 128:tt * 128 + 128],
                             rhs=tpv[:], start=False, stop=True)
            evict(ps1, tt, 1, nc.gpsimd)

        # channel 0: k-outer accumulation in two psum sweeps; the tiny
        # channel-1 groups fill the PE's DMA-pacing gaps
        def sweep(tts, ch1_sched, evict_queues):
            pss = {
                tt: ps0p.tile([128, 256], f32, tag="ps0", name=f"ps0_{tt}")
                for tt in tts
            }
            for k in range(8):
                for tt in tts:
                    for q in range(4):
                        off = tt * 128 + 3 - q
                        nc.tensor.matmul(
                            pss[tt][:],
                            lhsT=xs[k][:, off:off + 128],
                            rhs=aw[k][:, 256 * q:256 * (q + 1)],
                            start=(k == 0 and q == 0),
                            stop=(k == 7 and q == 3))
                for c1 in ch1_sched.get(k, []):
                    ch1_group(c1)
            for i, tt in enumerate(tts):
                evict(pss[tt], tt, 0, evict_queues[i % len(evict_queues)])

        sweep([0, 1, 2, 3, 4, 5], {k: [k - 2] for k in range(2, 8)},
              [nc.sync, nc.scalar])
        # sweep 2 runs tt-inner so tt6's eviction chain (DVE cast + DMA
        # issue + ~2us HBM write receipt) hides behind tt7's 32 matmuls,
        # leaving only tt7's own chain in the kernel tail.
        for tt, queue in ((6, nc.sync), (7, nc.scalar)):
            ch1_group(tt)
            ps = ps0p.tile([128, 256], f32, tag="ps0", name=f"ps0_{tt}")
            for k in range(8):
                for q in range(4):
                    off = tt * 128 + 3 - q
                    nc.tensor.matmul(
                        ps[:],
                        lhsT=xs[k][:, off:off + 128],
                        rhs=aw[k][:, 256 * q:256 * (q + 1)],
                        start=(k == 0 and q == 0),
                        stop=(k == 7 and q == 3))
            evict(ps, tt, 0, queue)

    nc.compile()
    return nc


def _inputs_for_cores(z: np.ndarray, window: np.ndarray):
    amat = _CACHE.get("amat")
    if amat is None:
        amat = _amat()
        _CACHE["amat"] = amat
    w64 = window.astype(np.float64)
    # window and the interior 1/ws (= 0.5) folded into A host-side
    awn = np.ascontiguousarray((amat * (w64 * 0.5)[None, :]).astype(BF16))
    # ch1 taps: rows 0-3 = w-quarters * 0.5/N, rows 4-7 = same * (-1)^r
    w4 = w64.reshape(4, 256) * (0.5 / N_FFT)
    alt = 1.0 - 2.0 * (np.arange(256) % 2)
    taps = np.ascontiguousarray(
        np.concatenate([w4, w4 * alt[None, :]], 0).astype(BF16))

    in_maps = []
    for c in range(N_CORES):
        G = 1024 * c - 1  # global frame index of slot 0
        X = np.zeros((1034, F_SLOTS), np.float32)
        lo, hi = max(0, G), min(T_FRAMES, G + F_SLOTS)
        s0, s1 = lo - G, hi - G
        X[0:513, s0:s1] = z[0, :, lo:hi]
        X[513:1024, s0:s1] = z[1, 1:512, lo:hi]
        X[1024, s0:s1] = z[1, 0, lo:hi]
        X[1025, s0:s1] = z[1, 512, lo:hi]
        for q in range(4):  # pre-shifted zi0/zi512 rows for the ch1 lhsT
            X[1026 + q, 0:NB] = X[1024, 3 - q:3 - q + NB]
            X[1030 + q, 0:NB] = X[1025, 3 - q:3 - q + NB]
        in_maps.append({
            "x": X.astype(BF16),
            "awn": awn,
            "taps": taps,
        })
    return in_maps


def kernel(z: np.ndarray, window: np.ndarray) -> np.ndarray:
    from concourse.bass_utils import run_bass_kernel_spmd

    z = np.asarray(z, dtype=np.float32)
    window = np.asarray(window, dtype=np.float32)

    nc = _CACHE.get("nc")
    if nc is None:
        nc = _build_nc()
        _CACHE["nc"] = nc

    in_maps = _inputs_for_cores(z, window)
    res = run_bass_kernel_spmd(nc, in_maps, list(range(N_CORES)))

    parts = []
    for c in range(N_CORES):
        nb = NB if c < N_CORES - 1 else NB - 1
        o = res.results[c]["out"]  # [2, NB, 256] bf16
        parts.append(o[:, :nb, :].reshape(2, -1).astype(np.float32))
    out = np.concatenate(parts, axis=1)
    # edge fixup: first/last 256 samples see a 3-frame window sum
    # (2 - w[768+r] and 2 - w[r]); the kernel normalized by 2 everywhere.
    w64 = window.astype(np.float64)
    out[:, :256] *= (2.0 / (2.0 - w64[768:1024])).astype(np.float32)
    out[:, -256:] *= (2.0 / (2.0 - w64[0:256])).astype(np.float32)
    return np.ascontiguousarray(out)
